# revision 8
# baseline (speedup 1.0000x reference)
"""Trainium2 Bass kernel for nn_CustomLoss_87522843558003 (YOLO-style CIoU+BCE loss).

Strategy (data-parallel over batch, 8 cores):
 - Each core processes 8 consecutive batches. Its 8*8400 positions map onto
   128 SBUF partitions as [batch(8) x section(16)] rows of 525 positions,
   processed in 3 chunks of L=175 positions.
 - Channel-interleaved SBUF layout (contiguous DMA); strided multi-dim APs
   do per-channel compute.
 - Anchor argmax via IoU cross-compare with fast reciprocal; selection with
   copy_predicated (first-max semantics); CIoU/BCE computed post-selection.
 - BCE uses exact-0/1 targets as a predication mask: bce = t ? ln(p) : ln(1-p).
 - Per-partition masked sums via accum_out -> tiny [128,12] output per core;
   final normalization on host.
Engine split: GPSIMD takes TT add/sub/mult bulk ops (no STT/max/min/cmp
support in Pool codegen); ACT takes relu/square/arctan/ln; DVE the rest.
"""

import numpy as np

B, A, N, CH = 64, 3, 8400, 15
NCORES = 8
BPC = B // NCORES      # batches per core
SEC = 16               # partition sections per batch
PPART = BPC * SEC      # 128 partitions
POSROW = N // SEC      # 525 positions per partition row
NCHUNK = 3
L = POSROW // NCHUNK   # 175 positions per chunk per row
C11 = CH - 4           # obj + cls channels
EPS = 1e-7

_CACHE = {}


def _build_bass(loop_r=None):
    """loop_r: if set, wrap the body in a device-side For_i loop repeating it
    loop_r times (identical work each pass; used for exec-time measurement)."""
    import contextlib
    import concourse.tile as tile
    import concourse.mybir as mybir
    from concourse import bacc

    Alu = mybir.AluOpType
    Act = mybir.ActivationFunctionType
    f32 = mybir.dt.float32

    nc = bacc.Bacc("TRN2", target_bir_lowering=False, debug=False,
                   num_devices=NCORES)
    predL = nc.dram_tensor("predL", [BPC, A, N, CH], f32, kind="ExternalInput").ap()
    targL = nc.dram_tensor("targL", [BPC, N, CH], f32, kind="ExternalInput").ap()
    accO = nc.dram_tensor("acc_out", [PPART, 12], f32, kind="ExternalOutput").ap()

    pre = predL.rearrange("b a (s k j) c -> b a s k (j c)", s=SEC, k=NCHUNK, j=L)
    tre = targL.rearrange("b (s k j) c -> b s k (j c)", s=SEC, k=NCHUNK, j=L)

    with tile.TileContext(nc) as tc:
        with (
            tc.tile_pool(name="pP", bufs=2) as pP,
            tc.tile_pool(name="pT", bufs=2) as pT,
            tc.tile_pool(name="pS", bufs=1) as pS,
            tc.tile_pool(name="pAcc", bufs=1) as pAcc,
        ):
            ACC = pAcc.tile([PPART, 12], f32)

            loop_cm = tc.For_i(0, loop_r, 1) if loop_r else contextlib.nullcontext()
            with loop_cm:
              for k in range(NCHUNK):
                P = pP.tile([PPART, A * L * CH], f32)
                T = pT.tile([PPART, L * CH], f32)
                for a in range(A):
                    nc.sync.dma_start(P[:, a * L * CH:(a + 1) * L * CH],
                                      pre[:, a, :, k, :])
                nc.sync.dma_start(T[:], tre[:, :, k, :])

                Pr = P[:].rearrange("p (a j c) -> p a j c", a=A, c=CH)
                Tr = T[:].rearrange("p (j c) -> p j c", c=CH)

                # ---- shared target prep ----
                TWHH = pS.tile([PPART, L * 2], f32)
                TLO = pS.tile([PPART, L * 2], f32)
                THI = pS.tile([PPART, L * 2], f32)
                TSUM = pS.tile([PPART, L * 2], f32)
                TA = pS.tile([PPART, L], f32)
                RTH = pS.tile([PPART, L], f32)
                RATIOT = pS.tile([PPART, L], f32)
                ATANT = pS.tile([PPART, L], f32)
                tlo = TLO[:].rearrange("p (j c) -> p j c", c=2)
                thi = THI[:].rearrange("p (j c) -> p j c", c=2)
                twhh = TWHH[:].rearrange("p (j c) -> p j c", c=2)
                nc.gpsimd.tensor_scalar(twhh, Tr[:, :, 2:4], 0.5, None, Alu.mult)
                nc.gpsimd.tensor_tensor(tlo, Tr[:, :, 0:2], twhh, Alu.subtract)
                nc.gpsimd.tensor_tensor(thi, Tr[:, :, 0:2], twhh, Alu.add)
                nc.gpsimd.tensor_tensor(TSUM[:], TLO[:], THI[:], Alu.add)
                nc.gpsimd.tensor_tensor(TA[:], Tr[:, :, 2], Tr[:, :, 3], Alu.mult)
                nc.vector.reciprocal_approx_fast(RTH[:], Tr[:, :, 3])
                nc.gpsimd.tensor_tensor(RATIOT[:], Tr[:, :, 2], RTH[:], Alu.mult)
                nc.scalar.activation(ATANT[:], RATIOT[:], Act.Arctan)

                # ---- per-anchor argmax path (all anchors fused per op) ----
                SS = pS.tile([PPART, A * L * 6], f32)
                SSr = SS[:].rearrange("p (a j c) -> p a j c", a=A, c=6)
                PWHH = pS.tile([PPART, A * L * 2], f32)
                LT = pS.tile([PPART, A * L * 2], f32)
                RB = pS.tile([PPART, A * L * 2], f32)
                WHR = pS.tile([PPART, A * L * 2], f32)
                WHC = pS.tile([PPART, A * L * 2], f32)
                pwhh = PWHH[:].rearrange("p (a j c) -> p a j c", a=A, c=2)
                ltr = LT[:].rearrange("p (a j c) -> p a j c", a=A, c=2)
                rbr = RB[:].rearrange("p (a j c) -> p a j c", a=A, c=2)
                whrr = WHR[:].rearrange("p (a j c) -> p a j c", a=A, c=2)
                whcr = WHC[:].rearrange("p (a j c) -> p a j c", a=A, c=2)
                tlob = tlo.unsqueeze(1).broadcast_to([PPART, A, L, 2])
                thib = thi.unsqueeze(1).broadcast_to([PPART, A, L, 2])

                nc.scalar.mul(pwhh, Pr[:, :, :, 2:4], 0.5)
                nc.gpsimd.tensor_tensor(SSr[:, :, :, 0:2], Pr[:, :, :, 0:2],
                                        pwhh, Alu.subtract)
                nc.gpsimd.tensor_tensor(SSr[:, :, :, 2:4], Pr[:, :, :, 0:2],
                                        pwhh, Alu.add)
                nc.vector.tensor_tensor(ltr, SSr[:, :, :, 0:2], tlob, Alu.max)
                nc.vector.tensor_tensor(rbr, SSr[:, :, :, 2:4], thib, Alu.min)
                # WHC = relu(rb - lt); sub on GPSIMD, relu on ACT
                nc.gpsimd.tensor_tensor(whrr, rbr, ltr, Alu.subtract)
                nc.scalar.activation(whcr, whrr, Act.Relu)

                PA = pS.tile([PPART, A * L], f32)
                S = pS.tile([PPART, A * L], f32)
                par = PA[:].rearrange("p (a j) -> p a j", a=A)
                sr = S[:].rearrange("p (a j) -> p a j", a=A)
                tab = TA[:].unsqueeze(1).broadcast_to([PPART, A, L])
                nc.gpsimd.tensor_tensor(par, Pr[:, :, :, 2], Pr[:, :, :, 3],
                                        Alu.mult)
                nc.gpsimd.tensor_tensor(sr, par, tab, Alu.add)
                nc.vector.tensor_tensor(SSr[:, :, :, 4], whcr[:, :, :, 0],
                                        whcr[:, :, :, 1], Alu.mult)
                nc.gpsimd.tensor_tensor(SSr[:, :, :, 5], sr, SSr[:, :, :, 4],
                                        Alu.subtract)

                # ---- argmax masks ----
                RU = pS.tile([PPART, A * L], f32)
                Q = pS.tile([PPART, A * L], f32)
                rur = RU[:].rearrange("p (a j) -> p a j", a=A)
                qr = Q[:].rearrange("p (a j) -> p a j", a=A)
                nc.vector.reciprocal_approx_fast(rur, SSr[:, :, :, 5])
                nc.vector.tensor_tensor(qr, SSr[:, :, :, 4], rur, Alu.mult)
                G2 = pS.tile([PPART, 2 * L], f32)
                G20 = pS.tile([PPART, L], f32)
                N21 = pS.tile([PPART, L], f32)
                W1 = pS.tile([PPART, L], f32)
                W2 = pS.tile([PPART, L], f32)
                g2r = G2[:].rearrange("p (g j) -> p g j", g=2)
                nc.vector.tensor_tensor(g2r, qr[:, 1:3], qr[:, 0:2], Alu.is_gt)
                nc.vector.tensor_tensor(G20[:], qr[:, 2], qr[:, 0], Alu.is_gt)
                nc.gpsimd.tensor_scalar(N21[:], g2r[:, 1], -1.0, 1.0,
                                        Alu.mult, Alu.add)
                nc.gpsimd.tensor_tensor(W1[:], g2r[:, 0], N21[:], Alu.mult)
                nc.gpsimd.tensor_tensor(W2[:], G20[:], g2r[:, 1], Alu.mult)

                # ---- selection (anchor0 blocks overwritten in place) ----
                w1i = W1[:].bitcast(mybir.dt.int32)
                w2i = W2[:].bitcast(mybir.dt.int32)
                w1b6 = w1i.unsqueeze(2).broadcast_to([PPART, L, 6])
                w2b6 = w2i.unsqueeze(2).broadcast_to([PPART, L, 6])
                nc.vector.copy_predicated(SSr[:, 0], w1b6, SSr[:, 1])
                nc.vector.copy_predicated(SSr[:, 0], w2b6, SSr[:, 2])
                w1b11 = w1i.unsqueeze(2).broadcast_to([PPART, L, C11])
                w2b11 = w2i.unsqueeze(2).broadcast_to([PPART, L, C11])
                nc.vector.copy_predicated(Pr[:, 0, :, 4:CH], w1b11, Pr[:, 1, :, 4:CH])
                nc.vector.copy_predicated(Pr[:, 0, :, 4:CH], w2b11, Pr[:, 2, :, 4:CH])

                SEL = SSr[:, 0]             # [p, j, 6]: x1 y1 x2 y2 i u
                SELP11 = Pr[:, 0, :, 4:CH]  # [p, j, 11]: obj+cls selected

                # ---- post-selection ciou ----
                DEN = pS.tile([PPART, L], f32)
                RIOU = pS.tile([PPART, L], f32)
                IOU = pS.tile([PPART, L], f32)
                OMI = pS.tile([PPART, L], f32)
                nc.vector.tensor_scalar(DEN[:], SEL[:, :, 5], EPS, None, Alu.add)
                nc.vector.reciprocal_approx_fast(RIOU[:], DEN[:])
                nc.gpsimd.tensor_tensor(IOU[:], SEL[:, :, 4], RIOU[:], Alu.mult)
                nc.vector.tensor_scalar(OMI[:], IOU[:], -1.0, 1.0, Alu.mult, Alu.add)

                CLO = pS.tile([PPART, L * 2], f32)
                CHI = pS.tile([PPART, L * 2], f32)
                CWHD = pS.tile([PPART, L * 4], f32)
                SQ = pS.tile([PPART, L * 4], f32)
                SP = pS.tile([PPART, L * 2], f32)
                SELWH = pS.tile([PPART, L * 2], f32)
                clor = CLO[:].rearrange("p (j c) -> p j c", c=2)
                chir = CHI[:].rearrange("p (j c) -> p j c", c=2)
                cwhdr = CWHD[:].rearrange("p (g j c) -> p g j c", g=2, c=2)
                sqr = SQ[:].rearrange("p (g j c) -> p g j c", g=2, c=2)
                spr = SP[:].rearrange("p (j c) -> p j c", c=2)
                selwhr = SELWH[:].rearrange("p (j c) -> p j c", c=2)
                nc.vector.tensor_tensor(clor, SEL[:, :, 0:2], tlo, Alu.min)
                nc.vector.tensor_tensor(chir, SEL[:, :, 2:4], thi, Alu.max)
                nc.gpsimd.tensor_tensor(cwhdr[:, 0], chir, clor, Alu.subtract)
                nc.gpsimd.tensor_tensor(spr, SEL[:, :, 0:2], SEL[:, :, 2:4],
                                        Alu.add)
                nc.gpsimd.tensor_tensor(
                    cwhdr[:, 1], spr, TSUM[:].rearrange("p (j c) -> p j c", c=2),
                    Alu.subtract)
                nc.scalar.square(SQ[:], CWHD[:])
                nc.gpsimd.tensor_tensor(selwhr, SEL[:, :, 2:4], SEL[:, :, 0:2],
                                        Alu.subtract)

                DIAG = pS.tile([PPART, L], f32)
                RDIAG = pS.tile([PPART, L], f32)
                CDR = pS.tile([PPART, L], f32)
                QD = pS.tile([PPART, L], f32)
                DIOU = pS.tile([PPART, L], f32)
                nc.vector.scalar_tensor_tensor(DIAG[:], sqr[:, 0, :, 0], EPS,
                                               sqr[:, 0, :, 1], Alu.add, Alu.add)
                nc.vector.reciprocal_approx_fast(RDIAG[:], DIAG[:])
                nc.gpsimd.tensor_tensor(CDR[:], sqr[:, 1, :, 0], sqr[:, 1, :, 1],
                                        Alu.add)
                nc.gpsimd.tensor_tensor(QD[:], CDR[:], RDIAG[:], Alu.mult)
                nc.vector.scalar_tensor_tensor(DIOU[:], QD[:], 0.25, OMI[:],
                                               Alu.mult, Alu.add)

                RH = pS.tile([PPART, L], f32)
                RATIO = pS.tile([PPART, L], f32)
                ATANP = pS.tile([PPART, L], f32)
                DV = pS.tile([PPART, L], f32)
                V = pS.tile([PPART, L], f32)
                nc.vector.reciprocal_approx_fast(RH[:], selwhr[:, :, 1])
                nc.gpsimd.tensor_tensor(RATIO[:], selwhr[:, :, 0], RH[:], Alu.mult)
                nc.scalar.activation(ATANP[:], RATIO[:], Act.Arctan)
                nc.gpsimd.tensor_tensor(DV[:], ATANT[:], ATANP[:], Alu.subtract)
                nc.scalar.activation(V[:], DV[:], Act.Square,
                                     scale=float(2.0 / np.pi))

                ADEN = pS.tile([PPART, L], f32)
                RADEN = pS.tile([PPART, L], f32)
                ALPHA = pS.tile([PPART, L], f32)
                AV = pS.tile([PPART, L], f32)
                CIOUP = pS.tile([PPART, L], f32)
                nc.vector.scalar_tensor_tensor(ADEN[:], V[:], EPS, OMI[:],
                                               Alu.add, Alu.add)
                nc.vector.reciprocal_approx_fast(RADEN[:], ADEN[:])
                nc.gpsimd.tensor_tensor(ALPHA[:], V[:], RADEN[:], Alu.mult)
                nc.gpsimd.tensor_tensor(AV[:], ALPHA[:], V[:], Alu.mult)
                nc.gpsimd.tensor_tensor(CIOUP[:], DIOU[:], AV[:], Alu.add)

                # ---- bce on selected obj+cls ----
                LOGP = pS.tile([PPART, L * C11], f32)
                LM = pS.tile([PPART, L * C11], f32)
                logpr = LOGP[:].rearrange("p (j c) -> p j c", c=C11)
                lmr = LM[:].rearrange("p (j c) -> p j c", c=C11)
                nc.scalar.activation(logpr, SELP11, Act.Ln)
                nc.scalar.activation(lmr, SELP11, Act.Ln, bias=1.0, scale=-1.0)
                t11i = T[:].bitcast(mybir.dt.int32).rearrange(
                    "p (j c) -> p j c", c=CH)[:, :, 4:CH]
                nc.vector.copy_predicated(lmr, t11i, logpr)

                # ---- masked accumulation (accum_out -> ACC columns) ----
                mask = Tr[:, :, 4]
                maskb = Tr[:, :, 4:5].broadcast_to([PPART, L, C11])
                B2s = pS.tile([PPART, L], f32)
                B3s = pS.tile([PPART, L], f32)
                CNTs = pS.tile([PPART, L], f32)
                nc.vector.scalar_tensor_tensor(logpr, lmr, -0.1, maskb,
                                               Alu.mult, Alu.mult,
                                               accum_out=ACC[:, k:k + 1])
                nc.vector.scalar_tensor_tensor(B2s[:], lmr[:, :, 0], -0.9, mask,
                                               Alu.mult, Alu.mult,
                                               accum_out=ACC[:, 3 + k:4 + k])
                nc.vector.scalar_tensor_tensor(B3s[:], CIOUP[:], 1.0, mask,
                                               Alu.mult, Alu.mult,
                                               accum_out=ACC[:, 6 + k:7 + k])
                nc.vector.tensor_scalar(CNTs[:], mask, 1.0, 0.0, Alu.mult,
                                        Alu.add, accum_out=ACC[:, 9 + k:10 + k])

            nc.sync.dma_start(accO, ACC[:])

    nc.compile()
    return nc


def kernel(pred, target):
    pred = np.ascontiguousarray(np.asarray(pred, dtype=np.float32))
    target = np.ascontiguousarray(np.asarray(target, dtype=np.float32))
    assert pred.shape == (B, A, N, CH) and target.shape == (B, N, CH)

    if "nc" not in _CACHE:
        _CACHE["nc"] = _build_bass()
    nc = _CACHE["nc"]

    from concourse import bass_utils

    in_maps = []
    for c in range(NCORES):
        lo, hi = c * BPC, (c + 1) * BPC
        in_maps.append({
            "predL": np.ascontiguousarray(pred[lo:hi]),
            "targL": np.ascontiguousarray(target[lo:hi]),
        })

    res = bass_utils.run_bass_kernel_spmd(nc, in_maps, core_ids=list(range(NCORES)))
    _CACHE["last_results"] = res

    per_batch_num = []
    per_batch_cnt = []
    for c in range(NCORES):
        acc = res.results[c]["acc_out"].astype(np.float32)   # [128, 12]
        s_part = acc[:, 0:9].sum(axis=1, dtype=np.float32)   # [128]
        c_part = acc[:, 9:12].sum(axis=1, dtype=np.float32)
        per_batch_num.append(s_part.reshape(BPC, SEC).sum(axis=1, dtype=np.float32))
        per_batch_cnt.append(c_part.reshape(BPC, SEC).sum(axis=1, dtype=np.float32))
    S_b = np.concatenate(per_batch_num).astype(np.float32)   # [64]
    C_b = np.concatenate(per_batch_cnt).astype(np.float32)
    loss = np.mean((S_b / C_b).astype(np.float32), dtype=np.float32)
    return np.float32(loss)
